# revision 1
# baseline (speedup 1.0000x reference)
"""Trainium2 Bass kernel for nn_Barrier_Net (DeepSet GNN message passing).

Strategy (8 NeuronCores, SPMD):
  - Each core owns 2048 contiguous agents (16 blocks of 128 agents).
  - Host slices the sorted edge list at agent-range boundaries and pads each
    128-agent block to a uniform C chunks of 128 edges, so the compiled
    program is identical on every core (pure SPMD, no collectives).
  - phi MLP runs in transposed layout (features on partitions, edges on the
    free dim); segment-sum is a one-hot matmul (one-hot built on-device with
    an is_equal tensor_scalar against an iota tile, using host-precomputed
    block-local ids; padded edges get id -1 so they contribute nothing).
  - bp3 is folded in as a rank-1 (degree x bp3) matmul into the aggregate.
  - rho runs data-parallel over the agent dim on the same core.
  - The barrier term and br3 (negligible FLOPs) are added on the host.
"""

import numpy as np

N_AGENTS = 16384
N_EDGES = 524288
N_CORES = 8
AG_PER_CORE = N_AGENTS // N_CORES  # 2048
BLK = 128                          # agents per block
NBLK = AG_PER_CORE // BLK          # 16 blocks per core
MARGIN = 1.2 * 0.15                # barrier margin

# matmul dtype mode: "f32" (safe), "f32r" (fast fp32, N>=256), "bf16"
MM_MODE = "f32r"
A_BUFS = 6
PSMLP_BUFS = 6
PSSM_BUFS = 1
PSAGG_BUFS = 1

_compiled = {}


def _build(C):
    """Build + schedule the SPMD Bass program for C 128-edge chunks/block."""
    from contextlib import ExitStack

    import concourse.bass as bass
    import concourse.tile as tile
    from concourse import bacc, mybir

    FP = mybir.dt.float32
    RDT = mybir.dt.float32r if MM_MODE == "f32r" else FP
    BF = mybir.dt.bfloat16
    E_BLK = C * 128                     # padded edges per block

    nc = bacc.Bacc("TRN2", target_bir_lowering=False, debug=False,
                   num_devices=N_CORES)

    def din(name, shape, dt=None):
        return nc.dram_tensor(name, shape, dt or FP,
                              kind="ExternalInput").ap()

    eT = din("eT", [4, NBLK * E_BLK], RDT)
    relT = din("relT", [128, NBLK * C])
    degT = din("degT", [1, AG_PER_CORE])
    iota = din("iota", [128, 128])
    ident = din("ident", [128, 128])
    Wp1 = din("Wp1", [4, 256], RDT)
    Wp2 = din("Wp2", [256, 256], RDT)
    Wp3 = din("Wp3", [256, 64], BF)
    Wr1 = din("Wr1", [64, 256], RDT)
    Wr2 = din("Wr2", [256, 256], RDT)
    Wr3 = din("Wr3", [256, 2])
    bp1 = din("bp1", [256, 1])
    bp2 = din("bp2", [256, 1])
    bp3 = din("bp3", [1, 64])
    br1 = din("br1", [256, 1])
    br2 = din("br2", [256, 1])
    out_d = nc.dram_tensor("out", [AG_PER_CORE, 2], FP,
                           kind="ExternalOutput").ap()

    RELU = mybir.ActivationFunctionType.Relu
    COPY = mybir.ActivationFunctionType.Copy
    EQ = mybir.AluOpType.is_equal
    ADD = mybir.AluOpType.add
    MAX = mybir.AluOpType.max

    with tile.TileContext(nc) as tc, ExitStack() as ctx:
        consts = ctx.enter_context(tc.tile_pool(name="consts", bufs=1))
        a_pool = ctx.enter_context(tc.tile_pool(name="acts", bufs=A_BUFS))
        ps_mlp = ctx.enter_context(
            tc.tile_pool(name="ps_mlp", bufs=PSMLP_BUFS, space="PSUM"))
        ps_sm = ctx.enter_context(
            tc.tile_pool(name="ps_sm", bufs=PSSM_BUFS, space="PSUM"))
        ps_agg = ctx.enter_context(
            tc.tile_pool(name="ps_agg", bufs=PSAGG_BUFS, space="PSUM"))

        def cload(name, ap, shape=None, dt=FP):
            t = consts.tile(shape or list(ap.shape), dt, tag=name)
            nc.sync.dma_start(t[:], ap)
            return t

        wp1_s = cload("wp1", Wp1, dt=RDT)
        wp2a_s = cload("wp2a", Wp2[0:128, :], dt=RDT)
        wp2b_s = cload("wp2b", Wp2[128:256, :], dt=RDT)
        wp3a_s = cload("wp3a", Wp3[0:128, :], dt=BF)
        wp3b_s = cload("wp3b", Wp3[128:256, :], dt=BF)
        wr1_s = cload("wr1", Wr1, dt=RDT)
        wr2a_s = cload("wr2a", Wr2[0:128, :], dt=RDT)
        wr2b_s = cload("wr2b", Wr2[128:256, :], dt=RDT)
        wr3a_s = cload("wr3a", Wr3[0:128, :])
        wr3b_s = cload("wr3b", Wr3[128:256, :])
        bp1a = cload("bp1a", bp1[0:128, :])
        bp1b = cload("bp1b", bp1[128:256, :])
        bp2a = cload("bp2a", bp2[0:128, :])
        bp2b = cload("bp2b", bp2[128:256, :])
        br1a = cload("br1a", br1[0:128, :])
        br1b = cload("br1b", br1[128:256, :])
        br2a = cload("br2a", br2[0:128, :])
        br2b = cload("br2b", br2[128:256, :])
        bp3_s = cload("bp3", bp3)
        iota_s = cload("iota", iota)
        ident_s = cload("ident", ident)
        relT_s = cload("relT", relT)
        degT_s = cload("degT", degT)
        aggT_s = consts.tile([64, AG_PER_CORE], RDT, tag="aggT")

        # chunk sizes per block: C//4 chunks of 512 edges + one tail
        sizes = [512] * (C // 4)
        if C % 4:
            sizes.append(128 * (C % 4))

        for j in range(NBLK):
            pagg = ps_agg.tile([128, 64], FP, tag="agg")
            first = True
            off = j * E_BLK
            sub = j * C
            for n in sizes:
                et = a_pool.tile([4, n], RDT, tag="et")
                nc.sync.dma_start(et[:], eT[:, off:off + n])
                ps1a = ps_mlp.tile([128, n], FP, tag="psmlp")
                ps1b = ps_mlp.tile([128, n], FP, tag="psmlp")
                nc.tensor.matmul(ps1a[:], wp1_s[:, 0:128],
                                 et[:], start=True, stop=True)
                nc.tensor.matmul(ps1b[:], wp1_s[:, 128:256],
                                 et[:], start=True, stop=True)
                h1a = a_pool.tile([128, n], RDT, tag="h1a")
                h1b = a_pool.tile([128, n], RDT, tag="h1b")
                nc.scalar.activation(h1a[:], ps1a[:], RELU, bias=bp1a[:, 0:1])
                nc.vector.tensor_scalar(h1b[:], ps1b[:], bp1b[:, 0:1], 0.0,
                                        ADD, MAX)
                ps2a = ps_mlp.tile([128, n], FP, tag="psmlp")
                ps2b = ps_mlp.tile([128, n], FP, tag="psmlp")
                nc.tensor.matmul(ps2a[:], wp2a_s[:, 0:128],
                                 h1a[:], start=True, stop=False)
                nc.tensor.matmul(ps2a[:], wp2b_s[:, 0:128],
                                 h1b[:], start=False, stop=True)
                nc.tensor.matmul(ps2b[:], wp2a_s[:, 128:256],
                                 h1a[:], start=True, stop=False)
                nc.tensor.matmul(ps2b[:], wp2b_s[:, 128:256],
                                 h1b[:], start=False, stop=True)
                h2a = a_pool.tile([128, n], BF, tag="h2a")
                h2b = a_pool.tile([128, n], BF, tag="h2b")
                nc.scalar.activation(h2a[:], ps2a[:], RELU, bias=bp2a[:, 0:1])
                nc.vector.tensor_scalar(h2b[:], ps2b[:], bp2b[:, 0:1], 0.0,
                                        ADD, MAX)
                nsub = n // 128
                ps3 = ps_sm.tile([128, 64 * nsub], FP, tag="sm")
                for s in range(nsub):
                    sl = slice(s * 128, (s + 1) * 128)
                    o3 = slice(s * 64, (s + 1) * 64)
                    nc.tensor.matmul(ps3[:, o3], h2a[:, sl], wp3a_s[:],
                                     start=True, stop=False)
                    nc.tensor.matmul(ps3[:, o3], h2b[:, sl], wp3b_s[:],
                                     start=False, stop=True)
                h3 = a_pool.tile([128, 64 * nsub], BF, tag="h3")
                nc.scalar.activation(h3[:], ps3[:], COPY)
                for s in range(nsub):
                    oh = a_pool.tile([128, 128], BF, tag="oh")
                    nc.gpsimd.tensor_scalar(oh[:], iota_s[:],
                                            relT_s[:, sub:sub + 1], None, EQ)
                    nc.tensor.matmul(pagg[:], oh[:], h3[:, s * 64:(s + 1) * 64],
                                     start=first, stop=False)
                    first = False
                    sub += 1
                off += n
            # fold in bp3: agg += deg (x) bp3   (rank-1)
            nc.tensor.matmul(pagg[:], degT_s[:, j * 128:(j + 1) * 128],
                             bp3_s[:], start=first, stop=True)
            agg_sb = a_pool.tile([128, 64], FP, tag="aggsb")
            nc.vector.tensor_copy(agg_sb[:], pagg[:])
            pst = ps_sm.tile([64, 128], FP, tag="sm")
            nc.tensor.transpose(pst[:], agg_sb[:], ident_s[:])
            nc.vector.tensor_copy(aggT_s[:, j * 128:(j + 1) * 128], pst[:])

        # rho: data-parallel over agents, 512 at a time
        for g in range(AG_PER_CORE // 512):
            sl = slice(g * 512, (g + 1) * 512)
            pr1a = ps_mlp.tile([128, 512], FP, tag="psmlp")
            pr1b = ps_mlp.tile([128, 512], FP, tag="psmlp")
            nc.tensor.matmul(pr1a[:], wr1_s[:, 0:128],
                             aggT_s[:, sl], start=True, stop=True)
            nc.tensor.matmul(pr1b[:], wr1_s[:, 128:256],
                             aggT_s[:, sl], start=True, stop=True)
            r1a = a_pool.tile([128, 512], RDT, tag="h1a")
            r1b = a_pool.tile([128, 512], RDT, tag="h1b")
            nc.scalar.activation(r1a[:], pr1a[:], RELU, bias=br1a[:, 0:1])
            nc.scalar.activation(r1b[:], pr1b[:], RELU, bias=br1b[:, 0:1])
            pr2a = ps_mlp.tile([128, 512], FP, tag="psmlp")
            pr2b = ps_mlp.tile([128, 512], FP, tag="psmlp")
            nc.tensor.matmul(pr2a[:], wr2a_s[:, 0:128],
                             r1a[:], start=True, stop=False)
            nc.tensor.matmul(pr2a[:], wr2b_s[:, 0:128],
                             r1b[:], start=False, stop=True)
            nc.tensor.matmul(pr2b[:], wr2a_s[:, 128:256],
                             r1a[:], start=True, stop=False)
            nc.tensor.matmul(pr2b[:], wr2b_s[:, 128:256],
                             r1b[:], start=False, stop=True)
            r2a = a_pool.tile([128, 512], FP, tag="h2a")
            r2b = a_pool.tile([128, 512], FP, tag="h2b")
            nc.scalar.activation(r2a[:], pr2a[:], RELU, bias=br2a[:, 0:1])
            nc.scalar.activation(r2b[:], pr2b[:], RELU, bias=br2b[:, 0:1])
            for s in range(4):
                pso = ps_sm.tile([128, 2], FP, tag="sm")
                ssl = slice(s * 128, (s + 1) * 128)
                nc.tensor.matmul(pso[:], r2a[:, ssl], wr3a_s[:],
                                 start=True, stop=False)
                nc.tensor.matmul(pso[:], r2b[:, ssl], wr3b_s[:],
                                 start=False, stop=True)
                o_sb = a_pool.tile([128, 2], FP, tag="osb")
                nc.vector.tensor_copy(o_sb[:], pso[:])
                nc.sync.dma_start(out_d[g * 512 + s * 128:
                                        g * 512 + (s + 1) * 128, :], o_sb[:])

    nc.compile()
    return nc


def _prep_inputs(edge_feats, segment_ids, ws):
    """Host-side shard + pad. Returns (C, in_maps)."""
    seg = np.asarray(segment_ids).astype(np.int64)
    ef = np.asarray(edge_feats, dtype=np.float32)
    bounds = np.searchsorted(seg, np.arange(0, N_AGENTS + 1, BLK))
    counts = np.diff(bounds)                      # edges per 128-agent block
    C = int(np.ceil(counts.max() / 128))
    E_BLK = C * 128

    iota = np.tile(np.arange(128, dtype=np.float32), (128, 1))
    ident = np.eye(128, dtype=np.float32)
    const_w = {
        "iota": iota, "ident": ident,
        "Wp1": ws["Wp1"], "Wp2": ws["Wp2"], "Wp3": ws["Wp3"],
        "Wr1": ws["Wr1"], "Wr2": ws["Wr2"], "Wr3": ws["Wr3"],
        "bp1": ws["bp1"].reshape(256, 1), "bp2": ws["bp2"].reshape(256, 1),
        "bp3": ws["bp3"].reshape(1, 64),
        "br1": ws["br1"].reshape(256, 1), "br2": ws["br2"].reshape(256, 1),
    }
    const_w = {k: np.ascontiguousarray(v, dtype=np.float32)
               for k, v in const_w.items()}
    import ml_dtypes
    const_w["Wp3"] = const_w["Wp3"].astype(ml_dtypes.bfloat16)

    in_maps = []
    for i in range(N_CORES):
        eT = np.zeros((4, NBLK * E_BLK), np.float32)
        relT = np.full((128, NBLK * C), -1.0, np.float32)
        deg = np.zeros(AG_PER_CORE, np.float32)
        for j in range(NBLK):
            g = NBLK * i + j
            s, e = bounds[g], bounds[g + 1]
            cnt = e - s
            eT[:, j * E_BLK: j * E_BLK + cnt] = ef[s:e].T
            rel = np.full(E_BLK, -1.0, np.float32)
            rel[:cnt] = (seg[s:e] - 128 * g).astype(np.float32)
            relT[:, j * C:(j + 1) * C] = rel.reshape(C, 128).T
            np.add.at(deg, seg[s:e] - AG_PER_CORE * i, 1.0)
        m = {"eT": eT, "relT": relT, "degT": deg.reshape(1, -1)}
        m.update(const_w)
        in_maps.append(m)
    return C, in_maps


def _host_barrier(edge_feats, segment_ids):
    ef = np.asarray(edge_feats, dtype=np.float64)
    seg = np.asarray(segment_ids).astype(np.int64)
    p = ef[:, :2]
    d = np.sqrt((p * p).sum(1, keepdims=True))
    contrib = -(p / d) / (d - MARGIN)
    barrier = np.zeros((N_AGENTS, 2), np.float64)
    np.add.at(barrier, seg, contrib)
    return barrier


def kernel(edge_feats, segment_ids, Wp1, bp1, Wp2, bp2, Wp3, bp3,
           Wr1, br1, Wr2, br2, Wr3, br3, _trace=False):
    from concourse.bass_utils import run_bass_kernel_spmd

    ws = dict(Wp1=Wp1, bp1=bp1, Wp2=Wp2, bp2=bp2, Wp3=Wp3, bp3=bp3,
              Wr1=Wr1, br1=br1, Wr2=Wr2, br2=br2, Wr3=Wr3, br3=br3)
    ws = {k: np.asarray(v, dtype=np.float32) for k, v in ws.items()}
    C, in_maps = _prep_inputs(edge_feats, segment_ids, ws)
    if C not in _compiled:
        _compiled[C] = _build(C)
    nc = _compiled[C]
    res = run_bass_kernel_spmd(nc, in_maps, list(range(N_CORES)),
                               trace=_trace)
    out = np.concatenate([res.results[i]["out"] for i in range(N_CORES)], 0)
    out = (out.astype(np.float64) + _host_barrier(edge_feats, segment_ids)
           + np.asarray(ws["br3"], np.float64).reshape(1, 2))
    if _trace:
        kernel._last_results = res
    return out.astype(np.float32)



# revision 28
# speedup vs baseline: 1.3203x; 1.3203x over previous
"""Trainium2 Bass kernel for nn_Barrier_Net (DeepSet GNN message passing).

Strategy (8 NeuronCores, SPMD):
  - Each core owns 2048 contiguous agents (16 blocks of 128 agents).
  - Host slices the sorted edge list at agent-range boundaries and pads each
    128-agent block to a uniform C chunks of 128 edges, so the compiled
    program is identical on every core (pure SPMD, no collectives).
  - phi runs transposed (features on partitions, edges free) in 256-edge
    chunks through a 9-stage software pipeline (one stage per iteration):
    L1 -> relu1 -> L2 -> relu2 -> L3 -> h3-drain -> segsum. Every PSUM
    stage tile is a single bank with a 2-deep ring, so ~9 chunks are in
    flight and no engine FIFO ever head-of-line blocks on a producer.
  - Both channel halves of a layer live in one [128, 2, n] PSUM tile and
    are drained by a single wide instruction; bp1 is folded into L1 via a
    ones-row so the merged drain needs no per-partition bias. h3 of two
    consecutive chunks is drained by one paired instruction.
  - PSUM->SBUF drains are greedily balanced between ACT and DVE at the
    instruction level.
  - Layer 2 (256x256, FLOP-dominant) optionally runs fp8-e4m3 DoubleRow
    (2 rows/cycle on PE). Weights pre-scaled by 32 (e4m3 normal range),
    folded out exactly via Wp3/32.
  - segment-sum is a one-hot matmul; one-hots are built on the otherwise
    idle GpSimd engine (padded edges get id -1 and contribute nothing).
  - bp3 folds in as a rank-1 (degree x bp3) matmul; rho runs data-parallel
    over agents, spread over iterations after every 4th block.
  - The barrier term and br3 (negligible FLOPs) are added on the host.
"""

import numpy as np

N_AGENTS = 16384
N_EDGES = 524288
N_CORES = 8
AG_PER_CORE = N_AGENTS // N_CORES  # 2048
BLK = 128                          # agents per block
NBLK = AG_PER_CORE // BLK          # 16 blocks per core
MARGIN = 1.2 * 0.15                # barrier margin

# L2 matmul mode: "fp8" (DoubleRow, 2x PE throughput) or "f32r" (safe)
L2_MODE = "f32r"
W2_SCALE = 32.0

_compiled = {}


def _build(C, zero_bp2):
    """Build + schedule the SPMD Bass program for C 128-edge chunks/block."""
    from contextlib import ExitStack

    import concourse.bass as bass
    import concourse.tile as tile
    from concourse import bacc, mybir

    FP = mybir.dt.float32
    RDT = mybir.dt.float32r
    BF = mybir.dt.bfloat16
    F8 = mybir.dt.float8e4
    HDT = F8 if L2_MODE == "fp8" else RDT
    E_BLK = C * 128                     # padded edges per block
    chunk_sizes = [256] * (C // 2) + ([128] if C % 2 else [])

    nc = bacc.Bacc("TRN2", target_bir_lowering=False, debug=False,
                   num_devices=N_CORES)

    def din(name, shape, dt=None):
        return nc.dram_tensor(name, shape, dt or FP,
                              kind="ExternalInput").ap()

    eT = din("eT", [5, NBLK * E_BLK], RDT)      # row 4 = ones (bp1 fold)
    # packed [128, x] f32 consts: iota | ident | wr3a | wr3b
    #   | bp2a | bp2b | br1a | br1b | br2a | br2b | relT
    PK = din("PK", [128, 266 + NBLK * C])
    PKR = din("PKR", [128, 512], RDT)           # wr2a | wr2b
    degT = din("degT", [1, AG_PER_CORE], BF)
    Wp1 = din("Wp1", [5, 256], RDT)             # row 4 = bp1
    W2F = din("W2F", [128, 4, 128],
              F8 if L2_MODE == "fp8" else RDT)  # w2a | w2b (k-tile split)
    Wp3 = din("Wp3", [128, 128], BF)            # wp3a | wp3b
    Wr1 = din("Wr1", [64, 256], RDT)
    bp3 = din("bp3", [1, 64], BF)
    # out[g, p, s, c] = output channel c of agent g*512 + s*128 + p
    out_d = nc.dram_tensor("out", [NBLK // 4, 128, 4, 2], FP,
                           kind="ExternalOutput").ap()

    RELU = mybir.ActivationFunctionType.Relu
    COPY = mybir.ActivationFunctionType.Copy
    EQ = mybir.AluOpType.is_equal
    ADD = mybir.AluOpType.add
    MAX = mybir.AluOpType.max
    DR = mybir.MatmulPerfMode.DoubleRow

    with tile.TileContext(nc) as tc, ExitStack() as ctx:
        consts = ctx.enter_context(tc.tile_pool(name="consts", bufs=1))
        et_pool = ctx.enter_context(tc.tile_pool(name="et", bufs=2))
        h_pool = ctx.enter_context(tc.tile_pool(name="acts", bufs=4))
        oh_pool = ctx.enter_context(tc.tile_pool(name="oh", bufs=12))
        sm_pool = ctx.enter_context(tc.tile_pool(name="small", bufs=3))
        ps1_pool = ctx.enter_context(
            tc.tile_pool(name="ps1", bufs=2, space="PSUM"))
        ps2_pool = ctx.enter_context(
            tc.tile_pool(name="ps2", bufs=2, space="PSUM"))
        ps3_pool = ctx.enter_context(
            tc.tile_pool(name="ps3", bufs=2, space="PSUM"))
        ps_sm = ctx.enter_context(
            tc.tile_pool(name="ps_sm", bufs=1, space="PSUM"))
        ps_agg = ctx.enter_context(
            tc.tile_pool(name="ps_agg", bufs=1, space="PSUM"))

        def cload(name, ap, dt=FP):
            t = consts.tile(list(ap.shape), dt, tag=name)
            nc.sync.dma_start(t[:], ap)
            return t

        wp1_s = cload("wp1", Wp1, dt=RDT)
        # front-load block 0's edge features so L1 starts ASAP
        etb = [et_pool.tile([5, E_BLK], RDT, tag="etb", name="etb")]
        nc.sync.dma_start(etb[0][:], eT[:, 0:E_BLK])
        pk_s = cload("pk", PK)
        w2f_s = cload("w2f", W2F, dt=F8 if L2_MODE == "fp8" else RDT)
        w2a_s = w2f_s[:, 0:2, :]
        w2b_s = w2f_s[:, 2:4, :]
        wp3f_s = cload("wp3f", Wp3, dt=BF)
        wp3a_s = wp3f_s[:, 0:64]
        wp3b_s = wp3f_s[:, 64:128]
        wr1_s = cload("wr1", Wr1, dt=RDT)
        bp3_s = cload("bp3", bp3, dt=BF)
        degT_s = cload("degT", degT, dt=BF)
        pkr_s = cload("pkr", PKR, dt=RDT)
        iota_s = pk_s[:, 0:128]
        ident_s = pk_s[:, 128:256]
        wr2a_s = pkr_s[:, 0:256]
        wr2b_s = pkr_s[:, 256:512]
        wr3a_s = pk_s[:, 256:258]
        wr3b_s = pk_s[:, 258:260]
        bp2a = pk_s[:, 260:261]
        bp2b = pk_s[:, 261:262]
        br1a = pk_s[:, 262:263]
        br1b = pk_s[:, 263:264]
        br2a = pk_s[:, 264:265]
        br2b = pk_s[:, 265:266]
        relT_s = pk_s[:, 266:]
        aggT_s = consts.tile([64, AG_PER_CORE], RDT, tag="aggT")

        # engine accumulators for greedy drain balancing: 0 = ACT, 1 = DVE
        acc = [0.0, 0.0]

        def drain(out, in_, relu, rows, bias=None):
            """PSUM->SBUF drain on the globally less-loaded engine."""
            ca = (rows + 222) * 0.8333
            cd = (rows + 120) * 1.0417
            e = 0 if acc[0] + ca <= acc[1] + cd else 1
            acc[e] += ca if e == 0 else cd
            if e == 0:
                nc.scalar.activation(out, in_, RELU if relu else COPY,
                                     bias=bias if bias is not None else 0.0)
            elif relu:
                if bias is not None:
                    nc.vector.tensor_scalar(out, in_, bias, 0.0, ADD, MAX)
                else:
                    nc.vector.tensor_scalar(out, in_, 0.0, None, MAX)
            else:
                nc.vector.tensor_copy(out, in_)

        def rho1(g):
            """rho stage 1 over agents [g*512, (g+1)*512)."""
            sl = slice(g * 512, (g + 1) * 512)
            pr1a = ps2_pool.tile([128, 2, 256], FP, tag="ps2")
            pr1b = ps3_pool.tile([128, 512], FP, tag="ps3")
            nc.tensor.matmul(pr1a[:, 0:2, :], wr1_s[:, 0:128],
                             aggT_s[:, sl], start=True, stop=True)
            nc.tensor.matmul(pr1b[:, :], wr1_s[:, 128:256],
                             aggT_s[:, sl], start=True, stop=True)
            return pr1a, pr1b

        def rho2(g, pr1a, pr1b):
            r1a = sm_pool.tile([128, 512], RDT, tag="r1a")
            r1b = sm_pool.tile([128, 512], RDT, tag="r1b")
            nc.scalar.activation(r1a[:], pr1a[:, 0:2, :], RELU,
                                 bias=br1a[:, 0:1])
            nc.vector.tensor_scalar(r1b[:], pr1b[:, :], br1b[:, 0:1],
                                    0.0, ADD, MAX)
            acc[0] += (512 + 222) * 0.8333
            acc[1] += (512 + 120) * 1.0417
            return r1a, r1b

        def rho3(g, r1a, r1b):
            pr2a = ps2_pool.tile([128, 2, 256], FP, tag="ps2")
            pr2b = ps3_pool.tile([128, 512], FP, tag="ps3")
            a2 = pr2a[:, 0:2, :]
            b2 = pr2b[:, :]
            nc.tensor.matmul(a2, wr2a_s[:, 0:128], r1a[:],
                             start=True, stop=False)
            nc.tensor.matmul(a2, wr2b_s[:, 0:128], r1b[:],
                             start=False, stop=True)
            nc.tensor.matmul(b2, wr2a_s[:, 128:256], r1a[:],
                             start=True, stop=False)
            nc.tensor.matmul(b2, wr2b_s[:, 128:256], r1b[:],
                             start=False, stop=True)
            return pr2a, pr2b

        def rho4(g, pr2a, pr2b):
            r2a = sm_pool.tile([128, 512], FP, tag="r2a")
            r2b = sm_pool.tile([128, 512], FP, tag="r2b")
            nc.scalar.activation(r2a[:], pr2a[:, 0:2, :], RELU,
                                 bias=br2a[:, 0:1])
            nc.vector.tensor_scalar(r2b[:], pr2b[:, :], br2b[:, 0:1],
                                    0.0, ADD, MAX)
            acc[0] += (512 + 222) * 0.8333
            acc[1] += (512 + 120) * 1.0417
            return r2a, r2b

        def rho5(g, r2a, r2b):
            pso = ps_sm.tile([128, 8], FP, tag="sm")
            for s in range(4):
                ssl = slice(s * 128, (s + 1) * 128)
                osl = slice(s * 2, (s + 1) * 2)
                nc.tensor.matmul(pso[:, osl], r2a[:, ssl], wr3a_s[:],
                                 start=True, stop=False)
                nc.tensor.matmul(pso[:, osl], r2b[:, ssl], wr3b_s[:],
                                 start=False, stop=True)
            return pso

        def rho6(g, pso):
            osb = sm_pool.tile([128, 8], FP, tag="osb")
            nc.vector.tensor_copy(osb[:], pso[:])
            acc[1] += (8 + 120) * 1.0417
            nc.sync.dma_start(out_d[g, :, :, :], osb[:])

        # flatten chunks: (block j, offset, n, first/last-in-block)
        chunks = []
        for j in range(NBLK):
            off = 0
            for ci, n in enumerate(chunk_sizes):
                chunks.append((j, off, n, ci == 0,
                               ci == len(chunk_sizes) - 1))
                off += n
        NCH = len(chunks)
        state = {}
        etb = [None]

        def s0(k):                       # L1 matmuls (+ block DMA)
            j, off, n, first, last = chunks[k]
            if first:
                etb[0] = et_pool.tile([5, E_BLK], RDT, tag="etb", name="etb")
                nc.sync.dma_start(etb[0][:],
                                  eT[:, j * E_BLK:(j + 1) * E_BLK])
            ps1 = ps1_pool.tile([128, 2, 256], FP, tag="ps1")
            esl = etb[0][:, off:off + n]
            nc.tensor.matmul(ps1[:, 0, 0:n], wp1_s[:, 0:128], esl,
                             start=True, stop=True)
            nc.tensor.matmul(ps1[:, 1, 0:n], wp1_s[:, 128:256], esl,
                             start=True, stop=True)
            state[("ps1", k)] = ps1

        def s1(k):                       # relu1 drain -> h1
            n = chunks[k][2]
            ps1 = state.pop(("ps1", k))
            h1 = h_pool.tile([128, 2, 256], HDT, tag="h1")
            drain(h1[:, 0:2, 0:n], ps1[:, 0:2, 0:n], True, 2 * n)
            state[("h1", k)] = h1

        def s2(k):                       # L2 matmuls
            n = chunks[k][2]
            h1 = state.pop(("h1", k))
            ps2 = ps2_pool.tile([128, 2, 256], FP, tag="ps2")
            if L2_MODE == "fp8":
                nc.tensor.matmul(ps2[:, 0, 0:n], w2a_s,
                                 h1[:, 0:2, 0:n], start=True, stop=True,
                                 perf_mode=DR)
                nc.tensor.matmul(ps2[:, 1, 0:n], w2b_s,
                                 h1[:, 0:2, 0:n], start=True, stop=True,
                                 perf_mode=DR)
            else:
                # complete each half's accumulation group before starting
                # the other: both halves share one PSUM bank and start=True
                # clears the whole bank's has_written bits
                for half, wh in ((0, w2a_s), (1, w2b_s)):
                    for kk in (0, 1):
                        nc.tensor.matmul(ps2[:, half, 0:n], wh[:, kk, :],
                                         h1[:, kk, 0:n], start=(kk == 0),
                                         stop=(kk == 1))
            state[("ps2", k)] = ps2

        def s3(k):                       # relu2 drain -> h2
            n = chunks[k][2]
            ps2 = state.pop(("ps2", k))
            h2 = h_pool.tile([128, 2, 256], BF, tag="h2")
            if zero_bp2:
                drain(h2[:, 0:2, 0:n], ps2[:, 0:2, 0:n], True, 2 * n)
            else:
                drain(h2[:, 0, 0:n], ps2[:, 0, 0:n], True, n,
                      bias=bp2a[:, 0:1])
                drain(h2[:, 1, 0:n], ps2[:, 1, 0:n], True, n,
                      bias=bp2b[:, 0:1])
            state[("h2", k)] = h2

        def s4(k):                       # L3 matmuls into paired ps3
            n = chunks[k][2]
            h2 = state.pop(("h2", k))
            if k % 2 == 0:
                state["ps3"] = ps3_pool.tile([128, 512], FP, tag="ps3",
                                             name="ps3")
                state["ps3w"] = 0
            ps3 = state["ps3"]
            base = state["ps3w"]
            nsub = n // 128
            for s in range(nsub):
                sl = slice(s * 128, (s + 1) * 128)
                o3 = slice(base + s * 64, base + (s + 1) * 64)
                nc.tensor.matmul(ps3[:, o3], h2[:, 0, sl], wp3a_s[:],
                                 start=True, stop=False)
                nc.tensor.matmul(ps3[:, o3], h2[:, 1, sl], wp3b_s[:],
                                 start=False, stop=True)
            state[("o3", k)] = (ps3, base, nsub)
            state["ps3w"] = base + nsub * 64

        def s5(k):                       # h3 drain (once per pair)
            if k % 2 == 0 and k + 1 < NCH:
                return                   # drained with its partner
            ps3, _, _ = state[("o3", k)]
            rows = state["ps3w"]
            h3 = h_pool.tile([128, 512], BF, tag="h3")
            drain(h3[:, 0:rows], ps3[:, 0:rows], False, rows)
            for kk in (k - 1, k) if k % 2 == 1 else (k,):
                p, base, nsub = state.pop(("o3", kk))
                state[("h3", kk)] = (h3, base, nsub)

        def s6(k):                       # one-hot + segsum (+ block close)
            j, off, n, first, last = chunks[k]
            h3, base, nsub = state.pop(("h3", k))
            if first:
                state["pagg"] = ps_agg.tile([128, 64], FP, tag="agg",
                                            name="pagg")
                state["first"] = True
            sub = j * C + off // 128
            pagg = state["pagg"]
            for s in range(nsub):
                oh = oh_pool.tile([128, 128], BF, tag="oh")
                nc.gpsimd.tensor_scalar(oh[:], iota_s[:],
                                        relT_s[:, sub:sub + 1], None, EQ)
                nc.tensor.matmul(pagg[:], oh[:],
                                 h3[:, base + s * 64:base + (s + 1) * 64],
                                 start=state.pop("first", False),
                                 stop=False)
                sub += 1
            if last:
                nc.tensor.matmul(pagg[:],
                                 degT_s[:, j * 128:(j + 1) * 128],
                                 bp3_s[:], start=False, stop=True)
                agg_sb = sm_pool.tile([128, 64], FP, tag="aggsb")
                nc.vector.tensor_copy(agg_sb[:], pagg[:])
                acc[1] += (64 + 120) * 1.0417
                pst = ps_sm.tile([64, 128], FP, tag="sm")
                nc.tensor.transpose(pst[:], agg_sb[:], ident_s[:])
                nc.vector.tensor_copy(
                    aggT_s[:, j * 128:(j + 1) * 128], pst[:])
                acc[1] += (128 + 120) * 1.0417
                if j % 4 == 3:
                    g = j // 4
                    stages = [rho1, rho2, rho3, rho4, rho5, rho6]

                    def step(i=0, args=(g,)):
                        out = stages[i](*args)
                        if i + 1 < len(stages):
                            if not isinstance(out, tuple):
                                out = (out,)
                            pending.append(
                                lambda: step(i + 1, (g,) + out))
                    pending.append(step)

        pending = []
        LAGS = [0, 1, 3, 4, 6, 7, 9]     # emission iteration of s0..s6
        phases = [s0, s1, s2, s3, s4, s5, s6]
        for it in range(NCH + LAGS[-1]):
            # emit later stages first so every consumer follows its producer
            for si in range(len(phases) - 1, -1, -1):
                k = it - LAGS[si]
                if 0 <= k < NCH:
                    phases[si](k)
            if pending:
                pending.pop(0)()
        while pending:
            pending.pop(0)()

    nc.compile()
    return nc


def _prep_inputs(edge_feats, segment_ids, ws):
    """Host-side shard + pad. Returns (C, zero_bp2, in_maps)."""
    import ml_dtypes

    seg = np.asarray(segment_ids).astype(np.int64)
    ef = np.asarray(edge_feats, dtype=np.float32)
    bounds = np.searchsorted(seg, np.arange(0, N_AGENTS + 1, BLK))
    counts = np.diff(bounds)                      # edges per 128-agent block
    C = int(np.ceil(counts.max() / 128))
    E_BLK = C * 128

    zero_bp2 = not np.any(ws["bp2"])
    wp1x = np.concatenate([ws["Wp1"], ws["bp1"].reshape(1, 256)], axis=0)
    if L2_MODE == "fp8":
        bp2 = ws["bp2"] * W2_SCALE
        wp3 = ws["Wp3"] / W2_SCALE
    else:
        bp2 = ws["bp2"]
        wp3 = ws["Wp3"]

    # packed [128, 266] head: iota | ident | wr3a | wr3b
    #   | bp2a | bp2b | br1a | br1b | br2a | br2b
    pk_head = np.concatenate([
        np.tile(np.arange(128, dtype=np.float32), (128, 1)),
        np.eye(128, dtype=np.float32),
        ws["Wr3"][0:128, :], ws["Wr3"][128:256, :],
        bp2[0:128].reshape(128, 1), bp2[128:256].reshape(128, 1),
        ws["br1"][0:128].reshape(128, 1), ws["br1"][128:256].reshape(128, 1),
        ws["br2"][0:128].reshape(128, 1), ws["br2"][128:256].reshape(128, 1),
    ], axis=1).astype(np.float32)
    pkr = np.ascontiguousarray(np.concatenate(
        [ws["Wr2"][0:128, :], ws["Wr2"][128:256, :]], axis=1),
        dtype=np.float32)

    const_w = {
        "Wp1": np.ascontiguousarray(wp1x, dtype=np.float32),
        "Wr1": np.ascontiguousarray(ws["Wr1"], dtype=np.float32),
        "bp3": np.ascontiguousarray(ws["bp3"].reshape(1, 64)
                                    ).astype(ml_dtypes.bfloat16),
        "Wp3": np.ascontiguousarray(
            np.concatenate([wp3[0:128, :], wp3[128:256, :]], axis=1)
        ).astype(ml_dtypes.bfloat16),
    }
    w2s = (ws["Wp2"] * (W2_SCALE if L2_MODE == "fp8" else 1.0)
           ).astype(np.float32)                          # [256, 256]
    # k-tile layout: w2a[k, t, m] = w2s[t*128 + k, m]
    w2i = w2s.reshape(2, 128, 256).transpose(1, 0, 2)    # [k, t, 256]
    w2f = np.ascontiguousarray(np.concatenate(
        [w2i[:, :, 0:128], w2i[:, :, 128:256]], axis=1))
    const_w["W2F"] = (w2f.astype(ml_dtypes.float8_e4m3)
                      if L2_MODE == "fp8" else w2f)

    in_maps = []
    for i in range(N_CORES):
        eTt = np.zeros((5, NBLK * E_BLK), np.float32)
        eTt[4, :] = 1.0
        relT = np.full((128, NBLK * C), -1.0, np.float32)
        deg = np.zeros(AG_PER_CORE, np.float32)
        for j in range(NBLK):
            g = NBLK * i + j
            s, e = bounds[g], bounds[g + 1]
            cnt = e - s
            eTt[0:4, j * E_BLK: j * E_BLK + cnt] = ef[s:e].T
            rel = np.full(E_BLK, -1.0, np.float32)
            rel[:cnt] = (seg[s:e] - 128 * g).astype(np.float32)
            relT[:, j * C:(j + 1) * C] = rel.reshape(C, 128).T
            np.add.at(deg, seg[s:e] - AG_PER_CORE * i, 1.0)
        m = {"eT": eTt,
             "degT": deg.reshape(1, -1).astype(ml_dtypes.bfloat16),
             "PKR": pkr,
             "PK": np.concatenate([pk_head, relT], axis=1)}
        m.update(const_w)
        in_maps.append(m)
    return C, zero_bp2, in_maps


def _host_barrier(edge_feats, segment_ids):
    ef = np.asarray(edge_feats, dtype=np.float64)
    seg = np.asarray(segment_ids).astype(np.int64)
    p = ef[:, :2]
    d = np.sqrt((p * p).sum(1, keepdims=True))
    contrib = -(p / d) / (d - MARGIN)
    barrier = np.zeros((N_AGENTS, 2), np.float64)
    np.add.at(barrier, seg, contrib)
    return barrier


def kernel(edge_feats, segment_ids, Wp1, bp1, Wp2, bp2, Wp3, bp3,
           Wr1, br1, Wr2, br2, Wr3, br3, _trace=False):
    from concourse.bass_utils import run_bass_kernel_spmd

    ws = dict(Wp1=Wp1, bp1=bp1, Wp2=Wp2, bp2=bp2, Wp3=Wp3, bp3=bp3,
              Wr1=Wr1, br1=br1, Wr2=Wr2, br2=br2, Wr3=Wr3, br3=br3)
    ws = {k: np.asarray(v, dtype=np.float32) for k, v in ws.items()}
    C, zero_bp2, in_maps = _prep_inputs(edge_feats, segment_ids, ws)
    key = (C, zero_bp2)
    if key not in _compiled:
        _compiled[key] = _build(C, zero_bp2)
    nc = _compiled[key]
    res = run_bass_kernel_spmd(nc, in_maps, list(range(N_CORES)),
                               trace=_trace)
    outs = []
    for i in range(N_CORES):
        o = res.results[i]["out"]          # [4, 128, 4, 2]
        outs.append(o.transpose(0, 2, 1, 3).reshape(AG_PER_CORE, 2))
    out = np.concatenate(outs, 0)
    out = (out.astype(np.float64) + _host_barrier(edge_feats, segment_ids)
           + np.asarray(ws["br3"], np.float64).reshape(1, 2))
    if _trace:
        kernel._last_results = res
    return out.astype(np.float32)


# revision 30
# speedup vs baseline: 1.3217x; 1.0011x over previous
"""Trainium2 Bass kernel for nn_Barrier_Net (DeepSet GNN message passing).

Strategy (8 NeuronCores, SPMD):
  - Each core owns 2048 contiguous agents (16 blocks of 128 agents).
  - Host slices the sorted edge list at agent-range boundaries and pads each
    128-agent block to a uniform C chunks of 128 edges, so the compiled
    program is identical on every core (pure SPMD, no collectives).
  - phi runs transposed (features on partitions, edges free) in 256-edge
    chunks through a 9-stage software pipeline (one stage per iteration):
    L1 -> relu1 -> L2 -> relu2 -> L3 -> h3-drain -> segsum. Every PSUM
    stage tile is a single bank with a 2-deep ring, so ~9 chunks are in
    flight and no engine FIFO ever head-of-line blocks on a producer.
  - Both channel halves of a layer live in one [128, 2, n] PSUM tile and
    are drained by a single wide instruction; bp1 is folded into L1 via a
    ones-row so the merged drain needs no per-partition bias. h3 of two
    consecutive chunks is drained by one paired instruction.
  - PSUM->SBUF drains are greedily balanced between ACT and DVE at the
    instruction level.
  - Layer 2 (256x256, FLOP-dominant) runs in fp32r (fp8-e4m3 DoubleRow is
    implemented behind L2_MODE but exceeds the accuracy budget: the barrier
    term partially cancels the MLP output, amplifying relative error).
  - Each layer's two PSUM halves share one bank, so each half's matmul
    accumulation group must close before the next opens (start=True clears
    the whole bank's has_written bits).
  - segment-sum is a one-hot matmul; one-hots are built on the otherwise
    idle GpSimd engine (padded edges get id -1 and contribute nothing).
  - bp3 folds in as a rank-1 (degree x bp3) matmul; rho runs data-parallel
    over agents, spread over iterations after every 4th block.
  - The barrier term and br3 (negligible FLOPs) are added on the host.
"""

import numpy as np

N_AGENTS = 16384
N_EDGES = 524288
N_CORES = 8
AG_PER_CORE = N_AGENTS // N_CORES  # 2048
BLK = 128                          # agents per block
NBLK = AG_PER_CORE // BLK          # 16 blocks per core
MARGIN = 1.2 * 0.15                # barrier margin

# L2 matmul mode: "fp8" (DoubleRow, 2x PE throughput) or "f32r" (safe)
L2_MODE = "f32r"
W2_SCALE = 32.0

_compiled = {}


def _build(C, zero_bp2):
    """Build + schedule the SPMD Bass program for C 128-edge chunks/block."""
    from contextlib import ExitStack

    import concourse.bass as bass
    import concourse.tile as tile
    from concourse import bacc, mybir

    FP = mybir.dt.float32
    RDT = mybir.dt.float32r
    BF = mybir.dt.bfloat16
    F8 = mybir.dt.float8e4
    HDT = F8 if L2_MODE == "fp8" else RDT
    E_BLK = C * 128                     # padded edges per block
    chunk_sizes = [256] * (C // 2) + ([128] if C % 2 else [])

    nc = bacc.Bacc("TRN2", target_bir_lowering=False, debug=False,
                   num_devices=N_CORES)

    def din(name, shape, dt=None):
        return nc.dram_tensor(name, shape, dt or FP,
                              kind="ExternalInput").ap()

    eT = din("eT", [5, NBLK * E_BLK], RDT)      # row 4 = ones (bp1 fold)
    # packed [128, x] f32 consts: iota | ident | wr3a | wr3b
    #   | bp2a | bp2b | br1a | br1b | br2a | br2b | relT
    PK = din("PK", [128, 266 + NBLK * C])
    PKR = din("PKR", [128, 512], RDT)           # wr2a | wr2b
    degT = din("degT", [1, AG_PER_CORE], BF)
    Wp1 = din("Wp1", [5, 256], RDT)             # row 4 = bp1
    W2F = din("W2F", [128, 4, 128],
              F8 if L2_MODE == "fp8" else RDT)  # w2a | w2b (k-tile split)
    Wp3 = din("Wp3", [128, 128], BF)            # wp3a | wp3b
    Wr1 = din("Wr1", [64, 256], RDT)
    bp3 = din("bp3", [1, 64], BF)
    # out[g, p, s, c] = output channel c of agent g*512 + s*128 + p
    out_d = nc.dram_tensor("out", [NBLK // 4, 128, 4, 2], FP,
                           kind="ExternalOutput").ap()

    RELU = mybir.ActivationFunctionType.Relu
    COPY = mybir.ActivationFunctionType.Copy
    EQ = mybir.AluOpType.is_equal
    ADD = mybir.AluOpType.add
    MAX = mybir.AluOpType.max
    DR = mybir.MatmulPerfMode.DoubleRow

    with tile.TileContext(nc) as tc, ExitStack() as ctx:
        consts = ctx.enter_context(tc.tile_pool(name="consts", bufs=1))
        et_pool = ctx.enter_context(tc.tile_pool(name="et", bufs=2))
        h_pool = ctx.enter_context(tc.tile_pool(name="acts", bufs=6))
        oh_pool = ctx.enter_context(tc.tile_pool(name="oh", bufs=16))
        sm_pool = ctx.enter_context(tc.tile_pool(name="small", bufs=4))
        ps1_pool = ctx.enter_context(
            tc.tile_pool(name="ps1", bufs=2, space="PSUM"))
        ps2_pool = ctx.enter_context(
            tc.tile_pool(name="ps2", bufs=2, space="PSUM"))
        ps3_pool = ctx.enter_context(
            tc.tile_pool(name="ps3", bufs=2, space="PSUM"))
        ps_sm = ctx.enter_context(
            tc.tile_pool(name="ps_sm", bufs=1, space="PSUM"))
        ps_agg = ctx.enter_context(
            tc.tile_pool(name="ps_agg", bufs=1, space="PSUM"))

        def cload(name, ap, dt=FP):
            t = consts.tile(list(ap.shape), dt, tag=name)
            nc.sync.dma_start(t[:], ap)
            return t

        wp1_s = cload("wp1", Wp1, dt=RDT)
        # front-load block 0's edge features so L1 starts ASAP
        etb = [et_pool.tile([5, E_BLK], RDT, tag="etb", name="etb")]
        nc.sync.dma_start(etb[0][:], eT[:, 0:E_BLK])
        pk_s = cload("pk", PK)
        w2f_s = cload("w2f", W2F, dt=F8 if L2_MODE == "fp8" else RDT)
        w2a_s = w2f_s[:, 0:2, :]
        w2b_s = w2f_s[:, 2:4, :]
        wp3f_s = cload("wp3f", Wp3, dt=BF)
        wp3a_s = wp3f_s[:, 0:64]
        wp3b_s = wp3f_s[:, 64:128]
        wr1_s = cload("wr1", Wr1, dt=RDT)
        bp3_s = cload("bp3", bp3, dt=BF)
        degT_s = cload("degT", degT, dt=BF)
        pkr_s = cload("pkr", PKR, dt=RDT)
        iota_s = pk_s[:, 0:128]
        ident_s = pk_s[:, 128:256]
        wr2a_s = pkr_s[:, 0:256]
        wr2b_s = pkr_s[:, 256:512]
        wr3a_s = pk_s[:, 256:258]
        wr3b_s = pk_s[:, 258:260]
        bp2a = pk_s[:, 260:261]
        bp2b = pk_s[:, 261:262]
        br1a = pk_s[:, 262:263]
        br1b = pk_s[:, 263:264]
        br2a = pk_s[:, 264:265]
        br2b = pk_s[:, 265:266]
        relT_s = pk_s[:, 266:]
        aggT_s = consts.tile([64, AG_PER_CORE], RDT, tag="aggT")

        # engine accumulators for greedy drain balancing: 0 = ACT, 1 = DVE
        acc = [0.0, 0.0]

        def drain(out, in_, relu, rows, bias=None):
            """PSUM->SBUF drain on the globally less-loaded engine."""
            ca = (rows + 222) * 0.8333
            cd = (rows + 120) * 1.0417
            e = 0 if acc[0] + ca <= acc[1] + cd else 1
            acc[e] += ca if e == 0 else cd
            if e == 0:
                nc.scalar.activation(out, in_, RELU if relu else COPY,
                                     bias=bias if bias is not None else 0.0)
            elif relu:
                if bias is not None:
                    nc.vector.tensor_scalar(out, in_, bias, 0.0, ADD, MAX)
                else:
                    nc.vector.tensor_scalar(out, in_, 0.0, None, MAX)
            else:
                nc.vector.tensor_copy(out, in_)

        def rho1(g):
            """rho stage 1 over agents [g*512, (g+1)*512)."""
            sl = slice(g * 512, (g + 1) * 512)
            pr1a = ps2_pool.tile([128, 2, 256], FP, tag="ps2")
            pr1b = ps3_pool.tile([128, 512], FP, tag="ps3")
            nc.tensor.matmul(pr1a[:, 0:2, :], wr1_s[:, 0:128],
                             aggT_s[:, sl], start=True, stop=True)
            nc.tensor.matmul(pr1b[:, :], wr1_s[:, 128:256],
                             aggT_s[:, sl], start=True, stop=True)
            return pr1a, pr1b

        def rho2(g, pr1a, pr1b):
            r1a = sm_pool.tile([128, 512], RDT, tag="r1a")
            r1b = sm_pool.tile([128, 512], RDT, tag="r1b")
            nc.scalar.activation(r1a[:], pr1a[:, 0:2, :], RELU,
                                 bias=br1a[:, 0:1])
            nc.vector.tensor_scalar(r1b[:], pr1b[:, :], br1b[:, 0:1],
                                    0.0, ADD, MAX)
            acc[0] += (512 + 222) * 0.8333
            acc[1] += (512 + 120) * 1.0417
            return r1a, r1b

        def rho3(g, r1a, r1b):
            pr2a = ps2_pool.tile([128, 2, 256], FP, tag="ps2")
            pr2b = ps3_pool.tile([128, 512], FP, tag="ps3")
            a2 = pr2a[:, 0:2, :]
            b2 = pr2b[:, :]
            nc.tensor.matmul(a2, wr2a_s[:, 0:128], r1a[:],
                             start=True, stop=False)
            nc.tensor.matmul(a2, wr2b_s[:, 0:128], r1b[:],
                             start=False, stop=True)
            nc.tensor.matmul(b2, wr2a_s[:, 128:256], r1a[:],
                             start=True, stop=False)
            nc.tensor.matmul(b2, wr2b_s[:, 128:256], r1b[:],
                             start=False, stop=True)
            return pr2a, pr2b

        def rho4(g, pr2a, pr2b):
            r2a = sm_pool.tile([128, 512], FP, tag="r2a")
            r2b = sm_pool.tile([128, 512], FP, tag="r2b")
            nc.scalar.activation(r2a[:], pr2a[:, 0:2, :], RELU,
                                 bias=br2a[:, 0:1])
            nc.vector.tensor_scalar(r2b[:], pr2b[:, :], br2b[:, 0:1],
                                    0.0, ADD, MAX)
            acc[0] += (512 + 222) * 0.8333
            acc[1] += (512 + 120) * 1.0417
            return r2a, r2b

        def rho5(g, r2a, r2b):
            pso = ps_sm.tile([128, 8], FP, tag="sm")
            for s in range(4):
                ssl = slice(s * 128, (s + 1) * 128)
                osl = slice(s * 2, (s + 1) * 2)
                nc.tensor.matmul(pso[:, osl], r2a[:, ssl], wr3a_s[:],
                                 start=True, stop=False)
                nc.tensor.matmul(pso[:, osl], r2b[:, ssl], wr3b_s[:],
                                 start=False, stop=True)
            return pso

        def rho6(g, pso):
            osb = sm_pool.tile([128, 8], FP, tag="osb")
            nc.vector.tensor_copy(osb[:], pso[:])
            acc[1] += (8 + 120) * 1.0417
            nc.sync.dma_start(out_d[g, :, :, :], osb[:])

        # flatten chunks: (block j, offset, n, first/last-in-block)
        chunks = []
        for j in range(NBLK):
            off = 0
            for ci, n in enumerate(chunk_sizes):
                chunks.append((j, off, n, ci == 0,
                               ci == len(chunk_sizes) - 1))
                off += n
        NCH = len(chunks)
        state = {}
        etb = [None]

        def s0(k):                       # L1 matmuls (+ block DMA)
            j, off, n, first, last = chunks[k]
            if first:
                etb[0] = et_pool.tile([5, E_BLK], RDT, tag="etb", name="etb")
                nc.sync.dma_start(etb[0][:],
                                  eT[:, j * E_BLK:(j + 1) * E_BLK])
            ps1 = ps1_pool.tile([128, 2, 256], FP, tag="ps1")
            esl = etb[0][:, off:off + n]
            nc.tensor.matmul(ps1[:, 0, 0:n], wp1_s[:, 0:128], esl,
                             start=True, stop=True)
            nc.tensor.matmul(ps1[:, 1, 0:n], wp1_s[:, 128:256], esl,
                             start=True, stop=True)
            state[("ps1", k)] = ps1

        def s1(k):                       # relu1 drain -> h1
            n = chunks[k][2]
            ps1 = state.pop(("ps1", k))
            h1 = h_pool.tile([128, 2, 256], HDT, tag="h1")
            drain(h1[:, 0:2, 0:n], ps1[:, 0:2, 0:n], True, 2 * n)
            state[("h1", k)] = h1

        def s2(k):                       # L2 matmuls
            n = chunks[k][2]
            h1 = state.pop(("h1", k))
            ps2 = ps2_pool.tile([128, 2, 256], FP, tag="ps2")
            if L2_MODE == "fp8":
                nc.tensor.matmul(ps2[:, 0, 0:n], w2a_s,
                                 h1[:, 0:2, 0:n], start=True, stop=True,
                                 perf_mode=DR)
                nc.tensor.matmul(ps2[:, 1, 0:n], w2b_s,
                                 h1[:, 0:2, 0:n], start=True, stop=True,
                                 perf_mode=DR)
            else:
                # complete each half's accumulation group before starting
                # the other: both halves share one PSUM bank and start=True
                # clears the whole bank's has_written bits
                for half, wh in ((0, w2a_s), (1, w2b_s)):
                    for kk in (0, 1):
                        nc.tensor.matmul(ps2[:, half, 0:n], wh[:, kk, :],
                                         h1[:, kk, 0:n], start=(kk == 0),
                                         stop=(kk == 1))
            state[("ps2", k)] = ps2

        def s3(k):                       # relu2 drain -> h2
            n = chunks[k][2]
            ps2 = state.pop(("ps2", k))
            h2 = h_pool.tile([128, 2, 256], BF, tag="h2")
            if zero_bp2:
                drain(h2[:, 0:2, 0:n], ps2[:, 0:2, 0:n], True, 2 * n)
            else:
                drain(h2[:, 0, 0:n], ps2[:, 0, 0:n], True, n,
                      bias=bp2a[:, 0:1])
                drain(h2[:, 1, 0:n], ps2[:, 1, 0:n], True, n,
                      bias=bp2b[:, 0:1])
            state[("h2", k)] = h2

        def s4(k):                       # L3 matmuls into paired ps3
            n = chunks[k][2]
            h2 = state.pop(("h2", k))
            if k % 2 == 0:
                state["ps3"] = ps3_pool.tile([128, 512], FP, tag="ps3",
                                             name="ps3")
                state["ps3w"] = 0
            ps3 = state["ps3"]
            base = state["ps3w"]
            nsub = n // 128
            for s in range(nsub):
                sl = slice(s * 128, (s + 1) * 128)
                o3 = slice(base + s * 64, base + (s + 1) * 64)
                nc.tensor.matmul(ps3[:, o3], h2[:, 0, sl], wp3a_s[:],
                                 start=True, stop=False)
                nc.tensor.matmul(ps3[:, o3], h2[:, 1, sl], wp3b_s[:],
                                 start=False, stop=True)
            state[("o3", k)] = (ps3, base, nsub)
            state["ps3w"] = base + nsub * 64

        def s5(k):                       # h3 drain (once per pair)
            if k % 2 == 0 and k + 1 < NCH:
                return                   # drained with its partner
            ps3, _, _ = state[("o3", k)]
            rows = state["ps3w"]
            h3 = h_pool.tile([128, 512], BF, tag="h3")
            drain(h3[:, 0:rows], ps3[:, 0:rows], False, rows)
            for kk in (k - 1, k) if k % 2 == 1 else (k,):
                p, base, nsub = state.pop(("o3", kk))
                state[("h3", kk)] = (h3, base, nsub)

        def s6(k):                       # one-hot + segsum (+ block close)
            j, off, n, first, last = chunks[k]
            h3, base, nsub = state.pop(("h3", k))
            if first:
                state["pagg"] = ps_agg.tile([128, 64], FP, tag="agg",
                                            name="pagg")
                state["first"] = True
            sub = j * C + off // 128
            pagg = state["pagg"]
            for s in range(nsub):
                oh = oh_pool.tile([128, 128], BF, tag="oh")
                nc.gpsimd.tensor_scalar(oh[:], iota_s[:],
                                        relT_s[:, sub:sub + 1], None, EQ)
                nc.tensor.matmul(pagg[:], oh[:],
                                 h3[:, base + s * 64:base + (s + 1) * 64],
                                 start=state.pop("first", False),
                                 stop=False)
                sub += 1
            if last:
                nc.tensor.matmul(pagg[:],
                                 degT_s[:, j * 128:(j + 1) * 128],
                                 bp3_s[:], start=False, stop=True)
                agg_sb = sm_pool.tile([128, 64], FP, tag="aggsb")
                nc.vector.tensor_copy(agg_sb[:], pagg[:])
                acc[1] += (64 + 120) * 1.0417
                pst = ps_sm.tile([64, 128], FP, tag="sm")
                nc.tensor.transpose(pst[:], agg_sb[:], ident_s[:])
                nc.vector.tensor_copy(
                    aggT_s[:, j * 128:(j + 1) * 128], pst[:])
                acc[1] += (128 + 120) * 1.0417
                if j % 4 == 3:
                    g = j // 4
                    stages = [rho1, rho2, rho3, rho4, rho5, rho6]

                    def step(i=0, args=(g,)):
                        out = stages[i](*args)
                        if i + 1 < len(stages):
                            if not isinstance(out, tuple):
                                out = (out,)
                            pending.append(
                                lambda: step(i + 1, (g,) + out))
                    pending.append(step)

        pending = []
        LAGS = [0, 1, 3, 4, 6, 7, 9]     # emission iteration of s0..s6
        phases = [s0, s1, s2, s3, s4, s5, s6]
        for it in range(NCH + LAGS[-1]):
            # emit later stages first so every consumer follows its producer
            for si in range(len(phases) - 1, -1, -1):
                k = it - LAGS[si]
                if 0 <= k < NCH:
                    phases[si](k)
            if pending:
                pending.pop(0)()
        while pending:
            pending.pop(0)()

    nc.compile()
    return nc


def _prep_inputs(edge_feats, segment_ids, ws):
    """Host-side shard + pad. Returns (C, zero_bp2, in_maps)."""
    import ml_dtypes

    seg = np.asarray(segment_ids).astype(np.int64)
    ef = np.asarray(edge_feats, dtype=np.float32)
    bounds = np.searchsorted(seg, np.arange(0, N_AGENTS + 1, BLK))
    counts = np.diff(bounds)                      # edges per 128-agent block
    C = int(np.ceil(counts.max() / 128))
    E_BLK = C * 128

    zero_bp2 = not np.any(ws["bp2"])
    wp1x = np.concatenate([ws["Wp1"], ws["bp1"].reshape(1, 256)], axis=0)
    if L2_MODE == "fp8":
        bp2 = ws["bp2"] * W2_SCALE
        wp3 = ws["Wp3"] / W2_SCALE
    else:
        bp2 = ws["bp2"]
        wp3 = ws["Wp3"]

    # packed [128, 266] head: iota | ident | wr3a | wr3b
    #   | bp2a | bp2b | br1a | br1b | br2a | br2b
    pk_head = np.concatenate([
        np.tile(np.arange(128, dtype=np.float32), (128, 1)),
        np.eye(128, dtype=np.float32),
        ws["Wr3"][0:128, :], ws["Wr3"][128:256, :],
        bp2[0:128].reshape(128, 1), bp2[128:256].reshape(128, 1),
        ws["br1"][0:128].reshape(128, 1), ws["br1"][128:256].reshape(128, 1),
        ws["br2"][0:128].reshape(128, 1), ws["br2"][128:256].reshape(128, 1),
    ], axis=1).astype(np.float32)
    pkr = np.ascontiguousarray(np.concatenate(
        [ws["Wr2"][0:128, :], ws["Wr2"][128:256, :]], axis=1),
        dtype=np.float32)

    const_w = {
        "Wp1": np.ascontiguousarray(wp1x, dtype=np.float32),
        "Wr1": np.ascontiguousarray(ws["Wr1"], dtype=np.float32),
        "bp3": np.ascontiguousarray(ws["bp3"].reshape(1, 64)
                                    ).astype(ml_dtypes.bfloat16),
        "Wp3": np.ascontiguousarray(
            np.concatenate([wp3[0:128, :], wp3[128:256, :]], axis=1)
        ).astype(ml_dtypes.bfloat16),
    }
    w2s = (ws["Wp2"] * (W2_SCALE if L2_MODE == "fp8" else 1.0)
           ).astype(np.float32)                          # [256, 256]
    # k-tile layout: w2a[k, t, m] = w2s[t*128 + k, m]
    w2i = w2s.reshape(2, 128, 256).transpose(1, 0, 2)    # [k, t, 256]
    w2f = np.ascontiguousarray(np.concatenate(
        [w2i[:, :, 0:128], w2i[:, :, 128:256]], axis=1))
    const_w["W2F"] = (w2f.astype(ml_dtypes.float8_e4m3)
                      if L2_MODE == "fp8" else w2f)

    in_maps = []
    for i in range(N_CORES):
        eTt = np.zeros((5, NBLK * E_BLK), np.float32)
        eTt[4, :] = 1.0
        relT = np.full((128, NBLK * C), -1.0, np.float32)
        deg = np.zeros(AG_PER_CORE, np.float32)
        for j in range(NBLK):
            g = NBLK * i + j
            s, e = bounds[g], bounds[g + 1]
            cnt = e - s
            eTt[0:4, j * E_BLK: j * E_BLK + cnt] = ef[s:e].T
            rel = np.full(E_BLK, -1.0, np.float32)
            rel[:cnt] = (seg[s:e] - 128 * g).astype(np.float32)
            relT[:, j * C:(j + 1) * C] = rel.reshape(C, 128).T
            np.add.at(deg, seg[s:e] - AG_PER_CORE * i, 1.0)
        m = {"eT": eTt,
             "degT": deg.reshape(1, -1).astype(ml_dtypes.bfloat16),
             "PKR": pkr,
             "PK": np.concatenate([pk_head, relT], axis=1)}
        m.update(const_w)
        in_maps.append(m)
    return C, zero_bp2, in_maps


def _host_barrier(edge_feats, segment_ids):
    ef = np.asarray(edge_feats, dtype=np.float64)
    seg = np.asarray(segment_ids).astype(np.int64)
    p = ef[:, :2]
    d = np.sqrt((p * p).sum(1, keepdims=True))
    contrib = -(p / d) / (d - MARGIN)
    barrier = np.zeros((N_AGENTS, 2), np.float64)
    np.add.at(barrier, seg, contrib)
    return barrier


def kernel(edge_feats, segment_ids, Wp1, bp1, Wp2, bp2, Wp3, bp3,
           Wr1, br1, Wr2, br2, Wr3, br3, _trace=False):
    from concourse.bass_utils import run_bass_kernel_spmd

    ws = dict(Wp1=Wp1, bp1=bp1, Wp2=Wp2, bp2=bp2, Wp3=Wp3, bp3=bp3,
              Wr1=Wr1, br1=br1, Wr2=Wr2, br2=br2, Wr3=Wr3, br3=br3)
    ws = {k: np.asarray(v, dtype=np.float32) for k, v in ws.items()}
    C, zero_bp2, in_maps = _prep_inputs(edge_feats, segment_ids, ws)
    key = (C, zero_bp2)
    if key not in _compiled:
        _compiled[key] = _build(C, zero_bp2)
    nc = _compiled[key]
    res = run_bass_kernel_spmd(nc, in_maps, list(range(N_CORES)),
                               trace=_trace)
    outs = []
    for i in range(N_CORES):
        o = res.results[i]["out"]          # [4, 128, 4, 2]
        outs.append(o.transpose(0, 2, 1, 3).reshape(AG_PER_CORE, 2))
    out = np.concatenate(outs, 0)
    out = (out.astype(np.float64) + _host_barrier(edge_feats, segment_ids)
           + np.asarray(ws["br3"], np.float64).reshape(1, 2))
    if _trace:
        kernel._last_results = res
    return out.astype(np.float32)
